# revision 8
# baseline (speedup 1.0000x reference)
"""Trainium2 Bass kernel for nn_AddDropMRR (add-drop microring resonator).

Math: rotate the complex plane per wavelength by -arg(G) (magnitudes are
invariant), where G = t2*s1/den is the ring response. With u = P*x + s*a:

  through^2 = (g*v)^2 + (c2*x)^2,   v = (r+P)*x + s*a,  g = |G|
  drop^2    = k2c^2*(u^2 + Q^2*x^2)

All per-wavelength coefficients depend only on `wavelengths` (8192 values)
and scalar params -> computed on HOST, DMA'd as tiny f32 tables. The device
graph is pure streaming with work spread over ALL FIVE engine queues
(measured per-[128,2048]-chunk costs):

  PE:    v = diag(r+P) @ x  (+)  I @ a'   (8 matmuls into PSUM, ~3.4us)
  ACT:   vv = (g/32 * v_psum)^2, 2x Sqrt, issues the 2 output DMAs (~7.4us)
  DVE:   u (TS-ptr 4x + TT 2x fp16), x^2, t1, t2, W2, D2 adds (~7.4us)
  GP:    u^2 (Pool/gpsimd tensor_mul, ~5.6us)
  DMA:   in on qSP ring, out on qAct ring (~6.9us saturated @ ~290GB/s)

Tensors ride fp16 (2-byte dtype enables DVE 2x/4x perf modes; better
mantissa than bf16); the /32, *1024 rescales keep (g*v)^2 under fp16 max.
Sharding: wavelength dim split 8 ways across cores, host-transposed so
wavelength lies on SBUF partitions; coefficients are [128,1] f32 scalars.
"""
import numpy as np

B = 2048           # batch
W = 8192           # wavelengths
NCORES = 8
WSH = W // NCORES  # 1024 wavelengths per core
P = 128            # SBUF partitions
NCHUNK = WSH // P  # 8 chunks per core
NCOEF = 4          # P, g/32, (c2/32)^2, Q^2
N_EFF = 2.4
CIRC = 2.0 * np.pi * 1e-05


def _host_prep(wavelengths, coupling_1, coupling_2, phi_1, phi_2, phi_ring,
               alpha):
    """Scalars, coefficient table [NCOEF, W] f32, diag blocks [W] (r+P)."""
    c1 = float(np.asarray(coupling_1).reshape(-1)[0])
    c2 = float(np.asarray(coupling_2).reshape(-1)[0])
    p1 = float(np.asarray(phi_1).reshape(-1)[0])
    pr = float(np.asarray(phi_ring).reshape(-1)[0])
    al = float(np.asarray(alpha).reshape(-1)[0])
    k1c = float(np.clip(c1, 0.01, 0.99))
    k2c = float(np.clip(c2, 0.01, 0.99))
    t1 = float(np.sqrt(1.0 - k1c * k1c))
    t2 = float(np.sqrt(1.0 - k2c * k2c))
    s = float(np.sqrt(c2))       # unclamped, as in reference
    s1 = float(np.sqrt(c1))      # unclamped
    kappa = float(al * np.sqrt(1.0 - c1 * c1) * np.sqrt(1.0 - c2 * c2))

    # phi in f32 exactly as the reference computes it, then f64 trig
    wl = np.asarray(wavelengths, np.float32)
    phi32 = (np.float32(2.0 * np.pi * N_EFF) / wl) * np.float32(CIRC) \
        + np.float32(pr)
    phi = phi32.astype(np.float64)
    sin_p = np.sin(phi + p1)
    cos_p = np.cos(phi + p1)
    sin_f = np.sin(phi)
    cos_f = np.cos(phi)

    Pv = -k1c * al * sin_p
    Qv = k1c * al * cos_p
    den_re = 1.0 - kappa * cos_f
    den2 = den_re * den_re + (kappa * sin_f) ** 2
    rsq = 1.0 / np.sqrt(den2)
    g = (t2 * s1) * rsq
    r = (t1 / (t2 * s1)) * den_re
    c2v = (t2 * s1 * Qv - t1 * kappa * sin_f) * rsq

    coefs = np.stack([
        Pv,
        g / 32.0,
        (c2v / 32.0) ** 2,
        Qv ** 2,
    ]).astype(np.float32)                       # [NCOEF, W]
    return coefs, dict(s=s, k2c=k2c, rp=(r + Pv).astype(np.float32))


def _build_graph(k2c, loop_n=1, nchunk=NCHUNK, bufs=6, pe=True, uueng="gp",
                 split_dma=True, vv_split=1):
    """SPMD per-core graph.
    pe: v via PE diag-matmul into PSUM (else DVE TS+TT like u).
    uueng: engine for u^2 ('gp' | 'dve').
    split_dma: stores issued from the ACT HWDGE ring instead of qSP.
    vv_split: number of ACT ops covering the vv square (PSUM read), 1 or 4.
    loop_n>1 wraps the body in an on-device For_i loop for timing."""
    import concourse.tile as tile
    from concourse import bacc, mybir, bass

    f32 = mybir.dt.float32
    f16 = mybir.dt.float16
    AF = mybir.ActivationFunctionType
    ALU = mybir.AluOpType

    wsh = nchunk * P
    nc = bacc.Bacc("TRN2", target_bir_lowering=False, debug=False,
                   num_devices=NCORES)
    x_ext = nc.declare_dram_parameter("x_t", [wsh, B], f16, isOutput=False)
    a_ext = nc.declare_dram_parameter("a_t", [wsh, B], f16, isOutput=False)
    cf_ext = nc.declare_dram_parameter("cf_t", [P, NCOEF * nchunk], f32,
                                       isOutput=False)
    dg_ext = nc.declare_dram_parameter("dg_t", [P, (nchunk + 1) * P], f16,
                                       isOutput=False)
    o1_ext = nc.declare_dram_parameter("o1_t", [wsh, B], f16, isOutput=True)
    o2_ext = nc.declare_dram_parameter("o2_t", [wsh, B], f16, isOutput=True)

    k2sq = float(k2c * k2c)
    out_eng = "scalar" if split_dma else "sync"

    with tile.TileContext(nc) as tc:
        with tc.tile_pool(name="cst", bufs=1) as cst, \
             tc.tile_pool(name="mio", bufs=bufs) as mio, \
             tc.tile_pool(name="psum", bufs=2,
                          space=bass.MemorySpace.PSUM) as psum:

            def body(_iv=None):
                cf = cst.tile([P, NCOEF * nchunk], f32, tag="cf", name="cf")
                nc.sync.dma_start(cf[:], cf_ext[:])
                dg = cst.tile([P, (nchunk + 1) * P], f16, tag="dg", name="dg")
                if pe:
                    nc.sync.dma_start(dg[:], dg_ext[:])

                def C(k, c):
                    return cf[:, k * nchunk + c:k * nchunk + c + 1]

                # Software-pipelined: 4 stages per chunk, issued with a skew
                # so no engine queue head ever waits on a same-round
                # cross-engine dependency (Tile preserves per-queue program
                # order; without the skew, chunk c+1's independent work sits
                # behind chunk c's stalled ops).
                st = {}

                def s0(c):
                    rs = slice(c * P, (c + 1) * P)
                    xt = mio.tile([P, B], f16, tag="xt", name="xt")
                    nc.sync.dma_start(xt[:], x_ext[rs, :])
                    at = mio.tile([P, B], f16, tag="at", name="at")
                    nc.sync.dma_start(at[:], a_ext[rs, :])
                    st[c] = dict(xt=xt, at=at)

                def s1(c):
                    d = st[c]
                    xt, at = d["xt"], d["at"]
                    if pe:
                        vps = psum.tile([P, B], f32, tag="vps", name="vps")
                        d["vps"] = vps
                        dgc = dg[:, c * P:(c + 1) * P]
                        ide = dg[:, nchunk * P:(nchunk + 1) * P]
                        for j in range(0, B, 512):
                            nc.tensor.matmul(vps[:, j:j + 512], dgc,
                                             xt[:, j:j + 512],
                                             start=True, stop=False)
                        for j in range(0, B, 512):
                            nc.tensor.matmul(vps[:, j:j + 512], ide,
                                             at[:, j:j + 512],
                                             start=False, stop=True)
                    ut = mio.tile([P, B], f16, tag="ut", name="ut")
                    nc.vector.tensor_scalar(ut[:], xt[:], C(0, c), None,
                                            ALU.mult)
                    nc.vector.tensor_add(ut[:], ut[:], at[:])
                    d["ut"] = ut
                    if not pe:
                        vt = mio.tile([P, B], f16, tag="vt", name="vt")
                        nc.vector.tensor_scalar(vt[:], xt[:], C(1, c), None,
                                                ALU.mult)
                        nc.vector.tensor_add(vt[:], vt[:], at[:])
                        d["vt"] = vt
                    # uu = u^2 (in place)
                    if uueng == "gp":
                        nc.gpsimd.tensor_mul(ut[:], ut[:], ut[:])
                    else:
                        nc.vector.tensor_mul(ut[:], ut[:], ut[:])

                def s2(c):
                    d = st[c]
                    xt = d["xt"]
                    # vv = (g/32 * v)^2
                    if pe:
                        vt = mio.tile([P, B], f16, tag="vt", name="vt")
                        d["vt"] = vt
                        if vv_split == 1:
                            nc.scalar.activation(vt[:], d["vps"][:],
                                                 AF.Square, scale=C(1, c))
                        else:
                            step = B // vv_split
                            for j in range(0, B, step):
                                nc.scalar.activation(
                                    vt[:, j:j + step],
                                    d["vps"][:, j:j + step],
                                    AF.Square, scale=C(1, c))
                    else:
                        vt = d["vt"]
                        nc.scalar.activation(vt[:], vt[:], AF.Square,
                                             scale=C(1, c))
                    # xx = x^2 (in place over xt); t1 = (c2/32)^2*xx
                    nc.vector.tensor_mul(xt[:], xt[:], xt[:])
                    w2 = mio.tile([P, B], f16, tag="w2", name="w2")
                    nc.vector.tensor_scalar(w2[:], xt[:], C(2, c), None,
                                            ALU.mult)
                    d["t1"] = w2

                def s3(c):
                    d = st.pop(c)
                    rs = slice(c * P, (c + 1) * P)
                    xt, ut, vt = d["xt"], d["ut"], d["vt"]
                    # W2 = vv + t1
                    nc.vector.tensor_add(vt[:], vt[:], d["t1"][:])
                    # t2 = Q^2*xx (in place); D2 = uu + t2
                    nc.vector.tensor_scalar(xt[:], xt[:], C(3, c), None,
                                            ALU.mult)
                    nc.vector.tensor_add(ut[:], ut[:], xt[:])
                    # through = sqrt(1024*W2'), drop = sqrt(k2c^2*D2)
                    nc.scalar.activation(vt[:], vt[:], AF.Sqrt, scale=1024.0)
                    nc.scalar.activation(ut[:], ut[:], AF.Sqrt, scale=k2sq)
                    getattr(nc, out_eng).dma_start(o1_ext[rs, :], vt[:])
                    getattr(nc, out_eng).dma_start(o2_ext[rs, :], ut[:])

                stages = [s0, s1, s2, s3]
                nstg = len(stages)
                for t in range(nchunk + nstg - 1):
                    for s in range(nstg - 1, -1, -1):
                        c = t - s
                        if 0 <= c < nchunk:
                            stages[s](c)

            if loop_n > 1:
                with tc.For_i(0, loop_n, 1):
                    body()
            else:
                body()

    nc.compile()
    return nc


def _shard_inputs(input_signal, add_signal, coefs, s, rp=None):
    x = np.asarray(input_signal, dtype=np.float32).astype(np.float16)
    a = (np.asarray(add_signal, dtype=np.float32)
         * np.float32(s)).astype(np.float16)
    in_maps = []
    for i in range(NCORES):
        sl = slice(i * WSH, (i + 1) * WSH)
        # coef layout [P, NCOEF*NCHUNK]: column k*NCHUNK+c holds coef k of
        # chunk c; row p is wavelength c*P+p within the shard.
        cf = np.ascontiguousarray(
            coefs[:, sl].reshape(NCOEF, NCHUNK, P)
            .transpose(2, 0, 1).reshape(P, NCOEF * NCHUNK))
        m = {
            "x_t": np.ascontiguousarray(x[:, sl].T),
            "a_t": np.ascontiguousarray(a[:, sl].T),
            "cf_t": cf,
        }
        # diag blocks: dg[:, c*P:(c+1)*P] = diag(rp[shard, chunk c]);
        # last block = identity.
        dgm = np.zeros((P, (NCHUNK + 1) * P), np.float16)
        if rp is not None:
            rsh = rp[sl].reshape(NCHUNK, P)
            for c in range(NCHUNK):
                dgm[:, c * P:(c + 1) * P] = np.diag(rsh[c].astype(np.float16))
        dgm[:, NCHUNK * P:] = np.eye(P, dtype=np.float16)
        m["dg_t"] = dgm
        in_maps.append(m)
    return in_maps


def _gather_outputs(results):
    through = np.empty((B, W), np.float32)
    drop = np.empty((B, W), np.float32)
    for i in range(NCORES):
        sl = slice(i * WSH, (i + 1) * WSH)
        through[:, sl] = results[i]["o1_t"].T.astype(np.float32)
        drop[:, sl] = results[i]["o2_t"].T.astype(np.float32)
    return through, drop


def kernel(input_signal, add_signal, wavelengths, coupling_1, coupling_2,
           phi_1, phi_2, phi_ring, alpha):
    from concourse.bass_utils import run_bass_kernel_spmd

    coefs, sc = _host_prep(wavelengths, coupling_1, coupling_2, phi_1, phi_2,
                           phi_ring, alpha)
    nc = _build_graph(sc["k2c"])
    in_maps = _shard_inputs(input_signal, add_signal, coefs, sc["s"],
                            rp=sc["rp"])
    res = run_bass_kernel_spmd(nc, in_maps, core_ids=list(range(NCORES)))
    return _gather_outputs(res.results)


# revision 9
# speedup vs baseline: 1.3962x; 1.3962x over previous
"""Trainium2 Bass kernel for nn_AddDropMRR (add-drop microring resonator).

Math: rotate the complex plane per wavelength by -arg(G) (magnitudes are
invariant), where G = t2*s1/den is the ring response. With u = P*x + s*a
and v = (r+P)*x + s*a:

  through = sqrt(g^2*v^2 + c2^2*x^2)        [all coefs per-wavelength]
  drop    = k2c * sqrt(u^2 + Q^2*x^2)

All per-wavelength coefficients depend only on `wavelengths` (8192 values)
and scalar params -> computed on HOST, DMA'd as small f32/f16 tables. The
device graph is pure streaming, software-pipelined in 5 skewed stages so no
engine queue blocks on a same-round cross-engine dependency:

  s0 DMA(qSP):  load x, a' chunk                      (~3.5us/chunk)
  s1 DVE:       u, v via TS-ptr (4x fp16) + TT (2x)   (~4.1us)
  s2 DVE:       u^2, v^2, x^2 in place                (~3.6us)
  s3 PE:        W2 = diag(g^2)@vv + diag(c2^2)@xx,
                D2 = diag(Q^2)@xx + I@uu  -> PSUM     (~5us, 16 matmuls)
  s4 ACT:       4x Sqrt([128,1024] PSUM half) -> fp16, output DMAs (qAct)

GPSIMD is deliberately idle: its SBUF port is shared with DVE's 2-port
perf modes, so gpsimd work serializes against the 4x TS ops (measured).
Tensors ride fp16 (2-byte dtype enables the DVE fast modes; better
mantissa than bf16). PSUM accumulates in f32 so no overflow rescaling is
needed. Sharding: wavelength dim split 8 ways across cores (data-parallel,
fully elementwise); host transposes so wavelength lies on SBUF partitions.
"""
import numpy as np

B = 2048           # batch
W = 8192           # wavelengths
NCORES = 8
WSH = W // NCORES  # 1024 wavelengths per core
P = 128            # SBUF partitions
NCHUNK = WSH // P  # 8 chunks per core
NCOEF = 5          # P, r+P, g/32, (c2/32)^2, Q^2
N_EFF = 2.4
CIRC = 2.0 * np.pi * 1e-05
MODE = "b"         # 'b': W2/D2 on PE;  'v': v on PE, adds on DVE


def _host_prep(wavelengths, coupling_1, coupling_2, phi_1, phi_2, phi_ring,
               alpha):
    """Scalars + per-wavelength coefficient vectors (f64 -> f32)."""
    c1 = float(np.asarray(coupling_1).reshape(-1)[0])
    c2 = float(np.asarray(coupling_2).reshape(-1)[0])
    p1 = float(np.asarray(phi_1).reshape(-1)[0])
    pr = float(np.asarray(phi_ring).reshape(-1)[0])
    al = float(np.asarray(alpha).reshape(-1)[0])
    k1c = float(np.clip(c1, 0.01, 0.99))
    k2c = float(np.clip(c2, 0.01, 0.99))
    t1 = float(np.sqrt(1.0 - k1c * k1c))
    t2 = float(np.sqrt(1.0 - k2c * k2c))
    s = float(np.sqrt(c2))       # unclamped, as in reference
    s1 = float(np.sqrt(c1))      # unclamped
    kappa = float(al * np.sqrt(1.0 - c1 * c1) * np.sqrt(1.0 - c2 * c2))

    # phi in f32 exactly as the reference computes it, then f64 trig
    wl = np.asarray(wavelengths, np.float32)
    phi32 = (np.float32(2.0 * np.pi * N_EFF) / wl) * np.float32(CIRC) \
        + np.float32(pr)
    phi = phi32.astype(np.float64)
    sin_p = np.sin(phi + p1)
    cos_p = np.cos(phi + p1)
    sin_f = np.sin(phi)
    cos_f = np.cos(phi)

    Pv = -k1c * al * sin_p
    Qv = k1c * al * cos_p
    den_re = 1.0 - kappa * cos_f
    den2 = den_re * den_re + (kappa * sin_f) ** 2
    rsq = 1.0 / np.sqrt(den2)
    g = (t2 * s1) * rsq
    r = (t1 / (t2 * s1)) * den_re
    c2v = (t2 * s1 * Qv - t1 * kappa * sin_f) * rsq

    coefs = np.stack([
        Pv,
        r + Pv,
        g / 32.0,
        (c2v / 32.0) ** 2,
        Qv ** 2,
    ]).astype(np.float32)                       # [NCOEF, W]
    vecs = dict(g2=(g * g), c22=(c2v * c2v), q2=(Qv * Qv), rp=(r + Pv))
    return coefs, dict(s=s, k2c=k2c, **{k: v.astype(np.float32)
                                        for k, v in vecs.items()})


def _build_graph(k2c, loop_n=1, nchunk=NCHUNK, bufs=8, mode=MODE,
                 split_dma=True):
    """SPMD per-core graph; see module docstring. loop_n>1 wraps the body
    in an on-device For_i loop for steady-state timing."""
    import concourse.tile as tile
    from concourse import bacc, mybir, bass

    f32 = mybir.dt.float32
    f16 = mybir.dt.float16
    AF = mybir.ActivationFunctionType
    ALU = mybir.AluOpType

    wsh = nchunk * P
    ndiag = (3 * nchunk + 1) if mode == "b" else (nchunk + 1)
    nc = bacc.Bacc("TRN2", target_bir_lowering=False, debug=False,
                   num_devices=NCORES)
    x_ext = nc.declare_dram_parameter("x_t", [wsh, B], f16, isOutput=False)
    a_ext = nc.declare_dram_parameter("a_t", [wsh, B], f16, isOutput=False)
    cf_ext = nc.declare_dram_parameter("cf_t", [P, NCOEF * nchunk], f32,
                                       isOutput=False)
    dg_ext = nc.declare_dram_parameter("dg_t", [P, ndiag * P], f16,
                                       isOutput=False)
    o1_ext = nc.declare_dram_parameter("o1_t", [wsh, B], f16, isOutput=True)
    o2_ext = nc.declare_dram_parameter("o2_t", [wsh, B], f16, isOutput=True)

    k2sq = float(k2c * k2c)
    out_eng = "scalar" if split_dma else "sync"
    HB = B // 2  # psum half width

    with tile.TileContext(nc) as tc:
        with tc.tile_pool(name="cst", bufs=1) as cst, \
             tc.tile_pool(name="mio", bufs=bufs) as mio, \
             tc.tile_pool(name="psum", bufs=2,
                          space=bass.MemorySpace.PSUM) as psum:

            def body(_iv=None):
                cf = cst.tile([P, NCOEF * nchunk], f32, tag="cf", name="cf")
                nc.sync.dma_start(cf[:], cf_ext[:])
                dg = cst.tile([P, ndiag * P], f16, tag="dg", name="dg")
                nc.sync.dma_start(dg[:], dg_ext[:])

                def C(k, c):
                    return cf[:, k * nchunk + c:k * nchunk + c + 1]

                def DG(k):  # k-th [P,P] diag block
                    return dg[:, k * P:(k + 1) * P]

                st = {}

                def s0(c):
                    rs = slice(c * P, (c + 1) * P)
                    xt = mio.tile([P, B], f16, tag="xt", name="xt")
                    nc.sync.dma_start(xt[:], x_ext[rs, :])
                    at = mio.tile([P, B], f16, tag="at", name="at")
                    nc.sync.dma_start(at[:], a_ext[rs, :])
                    st[c] = dict(xt=xt, at=at)

                def s1(c):
                    d = st[c]
                    xt, at = d["xt"], d["at"]
                    ut = mio.tile([P, B], f16, tag="ut", name="ut")
                    nc.vector.tensor_scalar(ut[:], xt[:], C(0, c), None,
                                            ALU.mult)
                    nc.vector.tensor_add(ut[:], ut[:], at[:])
                    d["ut"] = ut
                    if mode == "b":
                        vt = mio.tile([P, B], f16, tag="vt", name="vt")
                        nc.vector.tensor_scalar(vt[:], xt[:], C(1, c), None,
                                                ALU.mult)
                        nc.vector.tensor_add(vt[:], vt[:], at[:])
                        d["vt"] = vt
                    else:
                        vps = psum.tile([P, B], f32, tag="vps", name="vps")
                        d["vps"] = vps
                        for j in range(0, B, 512):
                            nc.tensor.matmul(vps[:, j:j + 512], DG(c),
                                             xt[:, j:j + 512],
                                             start=True, stop=False)
                        for j in range(0, B, 512):
                            nc.tensor.matmul(vps[:, j:j + 512], DG(nchunk),
                                             at[:, j:j + 512],
                                             start=False, stop=True)

                def s2(c):
                    d = st[c]
                    xt, ut = d["xt"], d["ut"]
                    nc.vector.tensor_mul(ut[:], ut[:], ut[:])   # uu
                    if mode == "b":
                        vt = d["vt"]
                        nc.vector.tensor_mul(vt[:], vt[:], vt[:])  # vv
                    else:
                        vt = mio.tile([P, B], f16, tag="vt", name="vt")
                        d["vt"] = vt
                        nc.scalar.activation(vt[:], d["vps"][:], AF.Square,
                                             scale=C(2, c))
                    nc.vector.tensor_mul(xt[:], xt[:], xt[:])   # xx

                def s3(c):
                    d = st[c]
                    xt, ut, vt = d["xt"], d["ut"], d["vt"]
                    if mode == "b":
                        # W2 halves: diag(g2) @ vv  (+)  diag(c22) @ xx
                        # D2 halves: diag(q2) @ xx  (+)  I @ uu
                        wps = [psum.tile([P, HB], f32, tag="wp", name="wp")
                               for _ in range(2)]
                        dps = [psum.tile([P, HB], f32, tag="dp", name="dp")
                               for _ in range(2)]
                        d["wps"], d["dps"] = wps, dps
                        mm = nc.tensor.matmul

                        def sweep(dst, dgb, src, start, stop):
                            for h in range(2):
                                for j in range(0, HB, 512):
                                    mm(dst[h][:, j:j + 512], dgb,
                                       src[:, h * HB + j:h * HB + j + 512],
                                       start=start, stop=stop)

                        sweep(wps, DG(c), vt, True, False)
                        sweep(wps, DG(nchunk + c), xt, False, True)
                        sweep(dps, DG(2 * nchunk + c), xt, True, False)
                        sweep(dps, DG(3 * nchunk), ut, False, True)
                    else:
                        w2 = mio.tile([P, B], f16, tag="w2", name="w2")
                        nc.vector.tensor_scalar(w2[:], xt[:], C(3, c), None,
                                                ALU.mult)
                        nc.vector.tensor_add(vt[:], vt[:], w2[:])
                        nc.vector.tensor_scalar(xt[:], xt[:], C(4, c), None,
                                                ALU.mult)
                        nc.vector.tensor_add(ut[:], ut[:], xt[:])

                def s4(c):
                    d = st.pop(c)
                    rs = slice(c * P, (c + 1) * P)
                    ut, vt = d["ut"], d["vt"]
                    if mode == "b":
                        for h in range(2):
                            hs = slice(h * HB, (h + 1) * HB)
                            nc.scalar.activation(vt[:, hs], d["wps"][h][:],
                                                 AF.Sqrt)
                            nc.scalar.activation(ut[:, hs], d["dps"][h][:],
                                                 AF.Sqrt, scale=k2sq)
                    else:
                        nc.scalar.activation(vt[:], vt[:], AF.Sqrt,
                                             scale=1024.0)
                        nc.scalar.activation(ut[:], ut[:], AF.Sqrt,
                                             scale=k2sq)
                    getattr(nc, out_eng).dma_start(o1_ext[rs, :], vt[:])
                    getattr(nc, out_eng).dma_start(o2_ext[rs, :], ut[:])

                stages = [s0, s1, s2, s3, s4]
                nstg = len(stages)
                for t in range(nchunk + nstg - 1):
                    for s in range(nstg - 1, -1, -1):
                        c = t - s
                        if 0 <= c < nchunk:
                            stages[s](c)

            if loop_n > 1:
                with tc.For_i(0, loop_n, 1):
                    body()
            else:
                body()

    nc.compile()
    return nc


def _shard_inputs(input_signal, add_signal, coefs, s, vecs=None, mode=MODE):
    x = np.asarray(input_signal, dtype=np.float32).astype(np.float16)
    a = (np.asarray(add_signal, dtype=np.float32)
         * np.float32(s)).astype(np.float16)
    vecs = vecs or {}
    in_maps = []
    for i in range(NCORES):
        sl = slice(i * WSH, (i + 1) * WSH)
        # coef layout [P, NCOEF*NCHUNK]: column k*NCHUNK+c holds coef k of
        # chunk c; row p is wavelength c*P+p within the shard.
        cf = np.ascontiguousarray(
            coefs[:, sl].reshape(NCOEF, NCHUNK, P)
            .transpose(2, 0, 1).reshape(P, NCOEF * NCHUNK))
        m = {
            "x_t": np.ascontiguousarray(x[:, sl].T),
            "a_t": np.ascontiguousarray(a[:, sl].T),
            "cf_t": cf,
        }

        def diag_blocks(names):
            nd = len(names) * NCHUNK + 1
            dgm = np.zeros((P, nd * P), np.float16)
            for k, nm in enumerate(names):
                vsh = np.asarray(vecs[nm])[sl].reshape(NCHUNK, P)
                for c in range(NCHUNK):
                    blk = k * NCHUNK + c
                    dgm[:, blk * P:(blk + 1) * P] = np.diag(
                        vsh[c].astype(np.float16))
            dgm[:, (nd - 1) * P:] = np.eye(P, dtype=np.float16)
            return dgm

        if mode == "b":
            m["dg_t"] = diag_blocks(["g2", "c22", "q2"])
        else:
            m["dg_t"] = diag_blocks(["rp"])
        in_maps.append(m)
    return in_maps


def _gather_outputs(results):
    through = np.empty((B, W), np.float32)
    drop = np.empty((B, W), np.float32)
    for i in range(NCORES):
        sl = slice(i * WSH, (i + 1) * WSH)
        through[:, sl] = results[i]["o1_t"].T.astype(np.float32)
        drop[:, sl] = results[i]["o2_t"].T.astype(np.float32)
    return through, drop


def kernel(input_signal, add_signal, wavelengths, coupling_1, coupling_2,
           phi_1, phi_2, phi_ring, alpha):
    from concourse.bass_utils import run_bass_kernel_spmd

    coefs, sc = _host_prep(wavelengths, coupling_1, coupling_2, phi_1, phi_2,
                           phi_ring, alpha)
    nc = _build_graph(sc["k2c"])
    in_maps = _shard_inputs(input_signal, add_signal, coefs, sc["s"], vecs=sc)
    res = run_bass_kernel_spmd(nc, in_maps, core_ids=list(range(NCORES)))
    return _gather_outputs(res.results)
